# revision 17
# baseline (speedup 1.0000x reference)
"""MiniSTU Trainium2 kernel (8 NeuronCores, Bass/Tile).

Math: the reference's FFT convolution + einsum collapses to
    y[b,l,o] = sum_g sum_{t<=l} phi_eff_g[l-t] * (x[b,t] @ M_g)[o]
over g in the 48 (filter k, sign) pairs, where phi_eff carries the
(-1)^s alternation for the minus branch (the two sgn factors in the
reference combine to (-1)^(l-t), i.e. an alternating filter).

Device algorithm per core (6 pairs per core, filter-dim sharding),
c-major pipeline over the 16 sequence tiles:
  stage 1 (f32r):  Z_g[c][t,(b,o)] = xT_tile.T @ M_g
  proj   (bf16):   W[cp] rows (p,r) = (P/alpha)^T Z_p[cp], cast fp8
  stage 2:         y[c] accumulates in ONE PSUM group:
                     far:  fp8 DoubleRow matmuls, two sequence tiles
                           (W[cp],W[cp+1]) + (G_d,G_{d-1}) per matmul
                     near: d=1 and d=0 dense Toeplitz blocks in bf16
                   then a single PSUM->SBUF copy and DMA out.
PSUM->SBUF casts round-robin over Vector/Scalar/GpSimd so no engine
is the bottleneck. The 8 per-core partial outputs are summed on host
(the gather for this sharding).
"""

import numpy as np

import concourse.bass as bass
import concourse.tile as tile
from concourse import mybir
from concourse.bass_utils import run_bass_kernel_spmd

L = 2048
K = 24
I = 256
O = 256
B = 2
TS = 128          # tile size along sequence
CT = L // TS      # 16 sequence tiles
NP = 6            # (k, sign) pairs per core
NPP = 3           # fused pair-pairs (filters) per core
N_CORES = 8
BO = B * O        # 512 fused (b, o) columns
RF = 32           # far-field rank, pairs 0-3 (pairs 4,5 get rank 64)
RW = 256          # alpha-folded basis columns: 4*32 + 2*64
ND = 14           # far G stacks (d = 2..15)
F32 = mybir.dt.float32
F32R = mybir.dt.float32r
BF16 = mybir.dt.bfloat16
F8 = mybir.dt.float8e4


# ---------------------------------------------------------------------------
# Workarounds for this container's walrus: it rejects any instruction that
# carries more than one sync-wait command.
# ---------------------------------------------------------------------------

def _prune_init_barrier(nc):
    """Drop the Bass-init all-engine EVSEM barrier and the unused const
    memsets from the 'main' bb (~3us of EVSEM latency before any work).
    Register init is per-engine; Tile emits its own sems for every
    cross-engine dependency, so the startup barrier guards nothing here."""
    for f in nc.m.functions:
        for blk in f.blocks:
            if blk.name != "main":
                continue
            keep = []
            for inst in blk.instructions:
                nm = type(inst).__name__
                if nm in ("InstMemset", "InstDrain", "InstEventSemaphore"):
                    continue
                keep.append(inst)
            blk.instructions = keep


def _split_sync_waits(nc, max_waits=1):
    """Hoist extra sem-waits onto same-engine NOPs inserted right before the
    offending instruction; queue order keeps the semantics identical."""
    for f in nc.m.functions:
        for blk in f.blocks:
            insts = list(blk.instructions)
            out = []
            changed = False
            for inst in insts:
                si = getattr(inst, "sync_info", None)
                waits = list(si.on_wait) if si is not None else []
                if len(waits) > max_waits:
                    changed = True
                    extra, keep = waits[:-max_waits], waits[-max_waits:]
                    for j in range(0, len(extra), max_waits):
                        nop = mybir.InstNoOp(
                            name=nc.get_next_instruction_name(), ins=[], outs=[]
                        )
                        nop.engine = inst.engine
                        nop.sync_info = mybir.SyncInfo(
                            on_wait=extra[j : j + max_waits], on_update=[]
                        )
                        out.append(nop)
                    inst.sync_info = mybir.SyncInfo(
                        on_wait=keep, on_update=list(si.on_update)
                    )
                out.append(inst)
            if changed:
                blk.instructions = out


class _TC(tile.TileContext):
    """TileContext whose tail skips the global drain barrier: every DMA's
    completion is awaited by its consumer, the output DMAs precede the
    drain on their queues, and nothing runs after this kernel."""

    def _drain_and_barrier(self, tick_clock, wait_clock):
        nc = self.nc
        nc.sync.drain()
        assert self.sems is not None
        popped = nc._tile_sem_poison_stack.pop()
        assert popped is self._sem_poison


# ---------------------------------------------------------------------------
# Device program (identical on all 8 cores; per-core data differs)
# ---------------------------------------------------------------------------

def _far_applies(co):
    """Far-field matmuls for output tile co: one fp8 DoubleRow matmul
    per source tile cp <= co-2, G stack gf[co-cp-2]. The 6 pairs' rank-32
    blocks sit at 32-aligned partition offsets across the two DoubleRow
    k-planes (pairs 0-3 in plane 0, pairs 4-5 in plane 1; the remaining
    plane-1 rows are zero in G)."""
    return [(cp, co - cp - 2) for cp in range(co - 1)]


def _wslot(p):
    """(plane, partition offset, rank) of pair p's block in the W stack.
    Pairs 4 and 5 get rank 64 so the two 128-row DoubleRow k-planes are
    fully written by the proj casts (no dead rows to zero)."""
    return (0, 32 * p, 32) if p < 4 else (1, 64 * (p - 4), 64)


def _pbcol(p):
    """Column offset of pair p's basis block in pb."""
    return 32 * p if p < 4 else 128 + 64 * (p - 4)


def _build_nc():
    nc = bass.Bass("TRN2", target_bir_lowering=False, debug=False,
                   num_devices=N_CORES)
    # x batched per sequence tile: [cp, i, (b, ic, t)]
    xT_d = nc.dram_tensor("xT", [CT, TS, B * 2 * TS], F32R, kind="ExternalInput")
    # M fused per filter: [pp, ic, i, (plus o | minus o)]
    m_d = nc.dram_tensor("m", [NPP, 2, TS, 2 * O], F32R, kind="ExternalInput")
    # dense Toeplitz blocks, diagonals 0..1: [d, t, (p, l)]
    tb_d = nc.dram_tensor("tb", [2, TS, NP * TS], BF16, kind="ExternalInput")
    # far basis, alpha-folded, per pair: [t, (p, r)]
    pb_d = nc.dram_tensor("pb", [TS, RW], BF16, kind="ExternalInput")
    # far G stacks: [i, part, ktile, l] (pair p rows per _wslot)
    gf_d = nc.dram_tensor("gf", [ND, TS, 2, TS], F8, kind="ExternalInput")
    yp_d = nc.dram_tensor("yp", [CT, TS, BO], F32, kind="ExternalOutput")

    with _TC(nc) as tc:
        with (
            tc.tile_pool(name="const", bufs=1) as cpool,
            tc.tile_pool(name="ys", bufs=3) as ypool,
            tc.tile_pool(name="ps1", bufs=4, space="PSUM") as ps1,
            tc.tile_pool(name="psW", bufs=2, space="PSUM") as psW,
            tc.tile_pool(name="ps2", bufs=2, space="PSUM") as ps2,
        ):
            xs = [cpool.tile([TS, B * 2 * TS], F32R, tag=f"x{cp}", name=f"x{cp}")
                  for cp in range(CT)]
            ms = [[cpool.tile([TS, 2 * O], F32R, tag=f"m{pp}{ic}", name=f"m{pp}{ic}")
                   for ic in range(2)] for pp in range(NPP)]
            tbs = [cpool.tile([TS, NP * TS], BF16, tag=f"t{d}", name=f"t{d}")
                   for d in range(2)]
            pbt = cpool.tile([TS, RW], BF16, tag="pb", name="pbt")
            gts = [cpool.tile([TS, 2, TS], F8, tag=f"g{i}", name=f"g{i}")
                   for i in range(ND)]
            # Z rings, one per filter: [t, pair-half, tile, (b, o)]
            zs = [cpool.tile([TS, 2, CT, BO], BF16, tag=f"z{pp}", name=f"z{pp}")
                  for pp in range(NPP)]
            # W ring: [part, ktile, tile, (b, o)] (pair p rows per _wslot)
            wt = cpool.tile([TS, 2, CT, BO], F8, tag="w", name="wt")

            # --- DMA prefetch: critical head spread over four queues, then
            # x tiles / G stacks interleaved by first-use order on sync.
            nc.sync.dma_start(xs[0][:, :2 * TS], xT_d[0][:, :2 * TS])
            nc.scalar.dma_start(ms[0][0][:], m_d[0, 0])
            nc.gpsimd.dma_start(ms[1][0][:], m_d[1, 0])
            nc.sync.dma_start(ms[2][0][:], m_d[2, 0])
            nc.scalar.dma_start(ms[0][1][:], m_d[0, 1])
            nc.gpsimd.dma_start(ms[1][1][:], m_d[1, 1])
            nc.sync.dma_start(xs[0][:, 2 * TS:], xT_d[0][:, 2 * TS:])
            nc.scalar.dma_start(ms[2][1][:], m_d[2, 1])
            nc.gpsimd.dma_start(tbs[0][:], tb_d[0])
            nc.scalar.dma_start(tbs[1][:], tb_d[1])
            nc.gpsimd.dma_start(pbt[:], pb_d[:])
            rest = [("x", cp) for cp in range(1, CT)]
            for i in range(ND):
                rest.insert(min(len(rest), 2 * i + 6), ("g", i))
            for kind, idx in rest:
                if kind == "x":
                    nc.sync.dma_start(xs[idx][:], xT_d[idx])
                else:
                    nc.sync.dma_start(gts[idx][:], gf_d[idx])

            # round-robin cast engines (GpSimd has no PSUM access on TRN2)
            engs = [nc.vector.tensor_copy, nc.scalar.copy]
            eng_i = [0]

            def cast(dst, src):
                engs[eng_i[0] % 2](dst, src)
                eng_i[0] += 1

            def stage1(it):
                """12 f32r matmuls -> Z[it] casts (bf16)."""
                for b in range(B):
                    pss = [ps1.tile([TS, 2, O], F32, tag="s1",
                                    name=f"ps{it}_{b}_{i}")
                           for i in range(NPP)]
                    for ic in range(2):
                        xchunk = xs[it][:, (b * 2 + ic) * TS:
                                        (b * 2 + ic + 1) * TS]
                        for pp in range(NPP):
                            nc.tensor.matmul(
                                pss[pp][:], xchunk, ms[pp][ic][:],
                                start=(ic == 0), stop=(ic == 1),
                            )
                    for pp in range(NPP):
                        cast(zs[pp][:, :, it, b * O:(b + 1) * O],
                             pss[pp][:])

            # Output tiles are processed in pairs (cA, cB) = (2k-2, 2k-1)
            # with two concurrent PSUM groups. The short fp8 DoubleRow far
            # matmuls are interleaved one-for-one between the long 512-col
            # stage-1/near matmuls so their weight loads overlap execution.
            for k in range(CT // 2 + 1):
                if k == 0:
                    stage1(0)
                    stage1(1)
                    stage1(2)
                    stage1(3)
                    continue
                cA, cB = 2 * k - 2, 2 * k - 1
                farA, farB = _far_applies(cA), _far_applies(cB)
                nA = len(farA) + (6 if cA >= 1 else 0) + 6
                nB = len(farB) + 12
                ypsA = ps2.tile([TS, BO], F32, tag="s2", name=f"ypsA{k}")
                ypsB = ps2.tile([TS, BO], F32, tag="s2", name=f"ypsB{k}")
                iA = [0]
                iB = [0]

                def mmA(lhsT, rhs, **kw):
                    nc.tensor.matmul(ypsA[:], lhsT, rhs, start=(iA[0] == 0),
                                     stop=(iA[0] == nA - 1), **kw)
                    iA[0] += 1

                def mmB(lhsT, rhs, **kw):
                    nc.tensor.matmul(ypsB[:], lhsT, rhs, start=(iB[0] == 0),
                                     stop=(iB[0] == nB - 1), **kw)
                    iB[0] += 1

                if 2 * k + 2 < CT:
                    stage1(2 * k + 2)
                    stage1(2 * k + 3)

                # far: cB's extra lowest-d apply alone, then (cA cp) with
                # (cB cp+1)
                dr = {"perf_mode": mybir.MatmulPerfMode.DoubleRow}
                if farB:
                    cp, gi = farB[0]
                    mmB(gts[gi][:], wt[:, :, cp, :], **dr)
                for cp, gi in farA:
                    mmA(gts[gi][:], wt[:, :, cp, :], **dr)
                    mmB(gts[gi][:], wt[:, :, cp + 1, :], **dr)
                # near d=1 interleaved with proj matmuls (the interleave
                # spaces each psW bank's reuse past its fp8 cast); then d=0.
                do_proj = cA <= CT - 3
                psws = []
                for p in range(NP):
                    if cA >= 1:
                        mmA(tbs[1][:, p * TS:(p + 1) * TS],
                            zs[p // 2][:, p % 2, cA - 1, :])
                    mmB(tbs[1][:, p * TS:(p + 1) * TS],
                        zs[p // 2][:, p % 2, cB - 1, :])
                    if do_proj:
                        rk = _wslot(p)[2]
                        for c in (cA, cB):
                            psw = psW.tile([64, BO], F32, tag="sW",
                                           name=f"psw{c}_{p}")
                            nc.tensor.matmul(
                                psw[:rk, :],
                                pbt[:, _pbcol(p):_pbcol(p) + rk],
                                zs[p // 2][:, p % 2, c, :],
                                start=True, stop=True,
                            )
                            psws.append((psw, p, c))
                            if len(psws) >= 3:
                                w, q, cq = psws[len(psws) - 3]
                                wj, wo, qk = _wslot(q)
                                cast(wt[wo:wo + qk, wj, cq, :], w[:qk, :])
                for p in range(NP):
                    mmA(tbs[0][:, p * TS:(p + 1) * TS],
                        zs[p // 2][:, p % 2, cA, :])
                    mmB(tbs[0][:, p * TS:(p + 1) * TS],
                        zs[p // 2][:, p % 2, cB, :])
                if do_proj:
                    for w, q, cq in psws[-2:]:
                        wj, wo, qk = _wslot(q)
                        cast(wt[wo:wo + qk, wj, cq, :], w[:qk, :])

                for c, yps in ((cA, ypsA), (cB, ypsB)):
                    ysb = ypool.tile([TS, BO], F32, tag="ysb",
                                     name=f"ysb{c}")
                    if k == CT // 2:
                        # tail: copy halves on both PSUM-capable engines
                        # and DMA each half as soon as its copy lands
                        nc.vector.tensor_copy(ysb[:, :O], yps[:, :O])
                        nc.gpsimd.dma_start(yp_d[c][:, :O], ysb[:, :O])
                        nc.scalar.copy(ysb[:, O:], yps[:, O:])
                        nc.scalar.dma_start(yp_d[c][:, O:], ysb[:, O:])
                    else:
                        cast(ysb[:], yps[:])
                        (nc.scalar if c % 2 else nc.gpsimd).dma_start(
                            yp_d[c], ysb[:])

    _prune_init_barrier(nc)
    _split_sync_waits(nc)
    return nc


# ---------------------------------------------------------------------------
# Host side: input staging, sharding, gather
# ---------------------------------------------------------------------------

def _build_toeplitz(phi_eff):
    """tb[g, d, t, l] = phi_eff[g, d*TS + l - t] (0 where the index is
    negative); phi_eff is [G, L]."""
    G = phi_eff.shape[0]
    pad = np.zeros((G, L + TS), np.float32)
    pad[:, TS:] = phi_eff
    d = np.arange(CT)[:, None, None]
    t = np.arange(TS)[None, :, None]
    l = np.arange(TS)[None, None, :]
    return pad[:, d * TS + l - t + TS]


def _prepare(x, phi, M_phi_plus, M_phi_minus):
    """Host prep: build per-core in_maps (no device execution)."""
    import ml_dtypes

    x = np.asarray(x, np.float32)
    phi = np.asarray(phi, np.float32)
    Mp = np.asarray(M_phi_plus, np.float32)
    Mm = np.asarray(M_phi_minus, np.float32)

    # [cp, i, (b, ic, t)]: per-sequence-tile chunks of x^T, one DMA per cp
    xT = np.ascontiguousarray(
        x.reshape(B, CT, TS, 2, TS).transpose(1, 4, 0, 3, 2)
    ).reshape(CT, TS, B * 2 * TS)
    sgn = ((-1.0) ** np.arange(L)).astype(np.float32)

    phi_eff = np.empty((2 * K, L), np.float32)
    for g in range(2 * K):
        k, s = g // 2, g % 2
        phi_eff[g] = phi[:, k] * (sgn if s else 1.0)
    tb_all = _build_toeplitz(phi_eff)  # [48, CT, TS, TS]

    # shared far-field t-basis over all pairs' far blocks (d >= 2)
    far_blocks = tb_all[:, 2:]
    gram = np.einsum("gdtl,gdsl->ts", far_blocks.astype(np.float64),
                     far_blocks.astype(np.float64))
    _, evec = np.linalg.eigh(gram)
    P = np.ascontiguousarray(evec[:, ::-1][:, :64]).astype(np.float32)
    G_all = np.einsum("tr,gdtl->gdrl", P, far_blocks)  # [48, 14, 64, TS]
    # per-(pair, r) fp8 range balancing: G rows scaled by alpha, the
    # basis columns (and so W rows) by 1/alpha
    maxG = np.abs(G_all).max(axis=(1, 3))  # [48, 64]
    alpha = np.clip(np.sqrt(8.0 / np.maximum(maxG, 1e-8)), 1.0 / 16, 8.0)
    G_s = G_all * alpha[:, None, :, None]

    nc = _build_nc()
    in_maps = []
    for core in range(N_CORES):
        gs = slice(core * NP, (core + 1) * NP)
        m_core = np.empty((NPP, 2, TS, 2 * O), np.float32)
        for j in range(NPP):
            k = NPP * core + j
            m_core[j, :, :, :O] = Mp[k].reshape(2, TS, O)
            m_core[j, :, :, O:] = Mm[k].reshape(2, TS, O)
        tb_core = np.ascontiguousarray(
            tb_all[gs, :2].transpose(1, 2, 0, 3)
        ).reshape(2, TS, NP * TS)
        g_core = G_s[gs]  # [6, 14, 64, TS]
        a_core = alpha[gs]  # [6, 64]
        pb_core = np.zeros((TS, RW), np.float32)
        gf_core = np.zeros((ND, TS, 2, TS), np.float32)
        for p in range(NP):
            wj, wo, rk = _wslot(p)
            c0 = _pbcol(p)
            pb_core[:, c0:c0 + rk] = P[:, :rk] / a_core[p, :rk]
            gf_core[:, wo:wo + rk, wj, :] = g_core[p, :, :rk, :]
        in_maps.append({
            "xT": xT,
            "m": m_core,
            "tb": tb_core.astype(ml_dtypes.bfloat16),
            "pb": pb_core.astype(ml_dtypes.bfloat16),
            "gf": gf_core.astype(ml_dtypes.float8_e4m3),
        })
    return nc, in_maps


def _gather(results):
    y = np.zeros((CT, TS, B, O), np.float64)
    for core in range(N_CORES):
        y += results[core]["yp"].reshape(CT, TS, B, O)
    return np.ascontiguousarray(
        y.transpose(2, 0, 1, 3).reshape(B, L, O)
    ).astype(np.float32)


def kernel(x, phi, M_phi_plus, M_phi_minus):
    nc, in_maps = _prepare(x, phi, M_phi_plus, M_phi_minus)
    res = run_bass_kernel_spmd(nc, in_maps, list(range(N_CORES)))
    return _gather(res.results)


# revision 19
# speedup vs baseline: 1.0422x; 1.0422x over previous
"""MiniSTU Trainium2 kernel (8 NeuronCores, Bass/Tile).

Math: the reference's FFT convolution + einsum collapses to
    y[b,l,o] = sum_g sum_{t<=l} phi_eff_g[l-t] * (x[b,t] @ M_g)[o]
over g in the 48 (filter k, sign) pairs, where phi_eff carries the
(-1)^s alternation for the minus branch (the two sgn factors in the
reference combine to (-1)^(l-t), i.e. an alternating filter).

Device algorithm per core (6 pairs per core, filter-dim sharding),
c-major pipeline over the 16 sequence tiles:
  stage 1 (f32r):  Z_g[c][t,(b,o)] = xT_tile.T @ M_g
  proj   (bf16):   W[cp] rows (p,r) = (P/alpha)^T Z_p[cp], cast fp8
  stage 2:         y[c] accumulates in ONE PSUM group:
                     far:  fp8 DoubleRow matmuls, two sequence tiles
                           (W[cp],W[cp+1]) + (G_d,G_{d-1}) per matmul
                     near: d=1 and d=0 dense Toeplitz blocks in bf16
                   then a single PSUM->SBUF copy and DMA out.
PSUM->SBUF casts round-robin over Vector/Scalar/GpSimd so no engine
is the bottleneck. The 8 per-core partial outputs are summed on host
(the gather for this sharding).
"""

import numpy as np

import concourse.bass as bass
import concourse.tile as tile
from concourse import mybir
from concourse.bass_utils import run_bass_kernel_spmd

L = 2048
K = 24
I = 256
O = 256
B = 2
TS = 128          # tile size along sequence
CT = L // TS      # 16 sequence tiles
NP = 6            # (k, sign) pairs per core
NPP = 3           # fused pair-pairs (filters) per core
N_CORES = 8
BO = B * O        # 512 fused (b, o) columns
RF = 32           # far-field rank, pairs 0-3 (pairs 4,5 get rank 64)
RW = 256          # alpha-folded basis columns: 4*32 + 2*64
ND = 14           # far G stacks (d = 2..15)
F32 = mybir.dt.float32
F32R = mybir.dt.float32r
BF16 = mybir.dt.bfloat16
F8 = mybir.dt.float8e4


# ---------------------------------------------------------------------------
# Workarounds for this container's walrus: it rejects any instruction that
# carries more than one sync-wait command.
# ---------------------------------------------------------------------------

def _prune_init_barrier(nc):
    """Drop the Bass-init all-engine EVSEM barrier and the unused const
    memsets from the 'main' bb (~3us of EVSEM latency before any work).
    Register init is per-engine; Tile emits its own sems for every
    cross-engine dependency, so the startup barrier guards nothing here."""
    for f in nc.m.functions:
        for blk in f.blocks:
            if blk.name != "main":
                continue
            keep = []
            for inst in blk.instructions:
                nm = type(inst).__name__
                if nm in ("InstMemset", "InstDrain", "InstEventSemaphore"):
                    continue
                keep.append(inst)
            blk.instructions = keep


def _split_sync_waits(nc, max_waits=1):
    """Hoist extra sem-waits onto same-engine NOPs inserted right before the
    offending instruction; queue order keeps the semantics identical."""
    for f in nc.m.functions:
        for blk in f.blocks:
            insts = list(blk.instructions)
            out = []
            changed = False
            for inst in insts:
                si = getattr(inst, "sync_info", None)
                waits = list(si.on_wait) if si is not None else []
                if len(waits) > max_waits:
                    changed = True
                    extra, keep = waits[:-max_waits], waits[-max_waits:]
                    for j in range(0, len(extra), max_waits):
                        nop = mybir.InstNoOp(
                            name=nc.get_next_instruction_name(), ins=[], outs=[]
                        )
                        nop.engine = inst.engine
                        nop.sync_info = mybir.SyncInfo(
                            on_wait=extra[j : j + max_waits], on_update=[]
                        )
                        out.append(nop)
                    inst.sync_info = mybir.SyncInfo(
                        on_wait=keep, on_update=list(si.on_update)
                    )
                out.append(inst)
            if changed:
                blk.instructions = out


class _TC(tile.TileContext):
    """TileContext whose tail skips the global drain barrier: every DMA's
    completion is awaited by its consumer, the output DMAs precede the
    drain on their queues, and nothing runs after this kernel."""

    def _drain_and_barrier(self, tick_clock, wait_clock):
        nc = self.nc
        nc.sync.drain()
        assert self.sems is not None
        popped = nc._tile_sem_poison_stack.pop()
        assert popped is self._sem_poison


# ---------------------------------------------------------------------------
# Device program (identical on all 8 cores; per-core data differs)
# ---------------------------------------------------------------------------

def _far_applies(co):
    """Far-field matmuls for output tile co: one fp8 DoubleRow matmul
    per source tile cp <= co-2, G stack gf[co-cp-2]. The 6 pairs' rank-32
    blocks sit at 32-aligned partition offsets across the two DoubleRow
    k-planes (pairs 0-3 in plane 0, pairs 4-5 in plane 1; the remaining
    plane-1 rows are zero in G)."""
    return [(cp, co - cp - 2) for cp in range(co - 1)]


def _wslot(p):
    """(plane, partition offset, rank) of pair p's block in the W stack.
    Pairs 4 and 5 get rank 64 so the two 128-row DoubleRow k-planes are
    fully written by the proj casts (no dead rows to zero)."""
    return (0, 32 * p, 32) if p < 4 else (1, 64 * (p - 4), 64)


def _pbcol(p):
    """Column offset of pair p's basis block in pb."""
    return 32 * p if p < 4 else 128 + 64 * (p - 4)


def _build_nc():
    nc = bass.Bass("TRN2", target_bir_lowering=False, debug=False,
                   num_devices=N_CORES)
    # x^T, i-major for chunked DMA: [i, cp, (b, ic, t)]
    xT_d = nc.dram_tensor("xT", [TS, CT, B * 2 * TS], BF16, kind="ExternalInput")
    # M fused per filter: [pp, ic, i, (plus o | minus o)]
    m_d = nc.dram_tensor("m", [NPP, 2, TS, 2 * O], BF16, kind="ExternalInput")
    # dense Toeplitz blocks, diagonals 0..1: [d, t, (p, l)]
    tb_d = nc.dram_tensor("tb", [2, TS, NP * TS], BF16, kind="ExternalInput")
    # far basis, alpha-folded, per pair: [t, (p, r)]
    pb_d = nc.dram_tensor("pb", [TS, RW], BF16, kind="ExternalInput")
    # far G stacks: [i, part, ktile, l] (pair p rows per _wslot)
    gf_d = nc.dram_tensor("gf", [ND, TS, 2, TS], F8, kind="ExternalInput")
    yp_d = nc.dram_tensor("yp", [CT, TS, BO], F32, kind="ExternalOutput")

    with _TC(nc) as tc:
        with (
            tc.tile_pool(name="const", bufs=1) as cpool,
            tc.tile_pool(name="ys", bufs=3) as ypool,
            tc.tile_pool(name="ps1", bufs=4, space="PSUM") as ps1,
            tc.tile_pool(name="psW", bufs=2, space="PSUM") as psW,
            tc.tile_pool(name="ps2", bufs=2, space="PSUM") as ps2,
        ):
            xbig = cpool.tile([TS, CT, B * 2 * TS], BF16, tag="x", name="xbig")
            ms = [[cpool.tile([TS, 2 * O], BF16, tag=f"m{pp}{ic}", name=f"m{pp}{ic}")
                   for ic in range(2)] for pp in range(NPP)]
            tbs = [cpool.tile([TS, NP * TS], BF16, tag=f"t{d}", name=f"t{d}")
                   for d in range(2)]
            pbt = cpool.tile([TS, RW], BF16, tag="pb", name="pbt")
            gts = [cpool.tile([TS, 2, TS], F8, tag=f"g{i}", name=f"g{i}")
                   for i in range(ND)]
            # Z rings, one per filter: [t, pair-half, tile, (b, o)]
            zs = [cpool.tile([TS, 2, CT, BO], BF16, tag=f"z{pp}", name=f"z{pp}")
                  for pp in range(NPP)]
            # W ring: [part, ktile, tile, (b, o)] (pair p rows per _wslot)
            wt = cpool.tile([TS, 2, CT, BO], F8, tag="w", name="wt")

            # --- DMA prefetch: critical head spread over four queues, then
            # x tiles / G stacks interleaved by first-use order on sync.
            nc.sync.dma_start(xbig[:, 0, :2 * TS], xT_d[:, 0, :2 * TS])
            nc.scalar.dma_start(ms[0][0][:], m_d[0, 0])
            nc.gpsimd.dma_start(ms[1][0][:], m_d[1, 0])
            nc.sync.dma_start(ms[2][0][:], m_d[2, 0])
            nc.scalar.dma_start(ms[0][1][:], m_d[0, 1])
            nc.gpsimd.dma_start(ms[1][1][:], m_d[1, 1])
            nc.sync.dma_start(xbig[:, 0, 2 * TS:], xT_d[:, 0, 2 * TS:])
            nc.scalar.dma_start(ms[2][1][:], m_d[2, 1])
            nc.gpsimd.dma_start(tbs[0][:], tb_d[0])
            nc.scalar.dma_start(tbs[1][:], tb_d[1])
            nc.gpsimd.dma_start(pbt[:], pb_d[:])
            # x in 4 chunked DMAs (fewer sync-queue issue slots), G stacks
            # interleaved after the chunks that unblock the next tiles
            nc.sync.dma_start(xbig[:, 1:4, :], xT_d[:, 1:4, :])
            nc.sync.dma_start(xbig[:, 4:8, :], xT_d[:, 4:8, :])
            nc.sync.dma_start(gts[0][:], gf_d[0])
            nc.sync.dma_start(gts[1][:], gf_d[1])
            nc.sync.dma_start(xbig[:, 8:12, :], xT_d[:, 8:12, :])
            nc.sync.dma_start(gts[2][:], gf_d[2])
            nc.sync.dma_start(gts[3][:], gf_d[3])
            nc.sync.dma_start(xbig[:, 12:16, :], xT_d[:, 12:16, :])
            for i in range(4, ND):
                nc.sync.dma_start(gts[i][:], gf_d[i])

            # round-robin cast engines (GpSimd has no PSUM access on TRN2)
            engs = [nc.vector.tensor_copy, nc.scalar.copy]
            eng_i = [0]

            def cast(dst, src):
                engs[eng_i[0] % 2](dst, src)
                eng_i[0] += 1

            def stage1(it):
                """12 f32r matmuls -> Z[it] casts (bf16)."""
                for b in range(B):
                    pss = [ps1.tile([TS, 2, O], F32, tag="s1",
                                    name=f"ps{it}_{b}_{i}")
                           for i in range(NPP)]
                    for ic in range(2):
                        xchunk = xbig[:, it, (b * 2 + ic) * TS:
                                      (b * 2 + ic + 1) * TS]
                        for pp in range(NPP):
                            nc.tensor.matmul(
                                pss[pp][:], xchunk, ms[pp][ic][:],
                                start=(ic == 0), stop=(ic == 1),
                            )
                    for pp in range(NPP):
                        cast(zs[pp][:, :, it, b * O:(b + 1) * O],
                             pss[pp][:])

            # Output tiles are processed in pairs (cA, cB) = (2k-2, 2k-1)
            # with two concurrent PSUM groups. The short fp8 DoubleRow far
            # matmuls are interleaved one-for-one between the long 512-col
            # stage-1/near matmuls so their weight loads overlap execution.
            for k in range(CT // 2 + 1):
                if k == 0:
                    stage1(0)
                    stage1(1)
                    stage1(2)
                    stage1(3)
                    continue
                cA, cB = 2 * k - 2, 2 * k - 1
                farA, farB = _far_applies(cA), _far_applies(cB)
                nA = len(farA) + (6 if cA >= 1 else 0) + 6
                nB = len(farB) + 12
                ypsA = ps2.tile([TS, BO], F32, tag="s2", name=f"ypsA{k}")
                ypsB = ps2.tile([TS, BO], F32, tag="s2", name=f"ypsB{k}")
                iA = [0]
                iB = [0]

                def mmA(lhsT, rhs, **kw):
                    nc.tensor.matmul(ypsA[:], lhsT, rhs, start=(iA[0] == 0),
                                     stop=(iA[0] == nA - 1), **kw)
                    iA[0] += 1

                def mmB(lhsT, rhs, **kw):
                    nc.tensor.matmul(ypsB[:], lhsT, rhs, start=(iB[0] == 0),
                                     stop=(iB[0] == nB - 1), **kw)
                    iB[0] += 1

                if 2 * k + 2 < CT:
                    stage1(2 * k + 2)
                    stage1(2 * k + 3)

                # far: cB's extra lowest-d apply alone, then (cA cp) with
                # (cB cp+1)
                dr = {"perf_mode": mybir.MatmulPerfMode.DoubleRow}
                if farB:
                    cp, gi = farB[0]
                    mmB(gts[gi][:], wt[:, :, cp, :], **dr)
                for cp, gi in farA:
                    mmA(gts[gi][:], wt[:, :, cp, :], **dr)
                    mmB(gts[gi][:], wt[:, :, cp + 1, :], **dr)
                # near d=1 interleaved with proj matmuls (the interleave
                # spaces each psW bank's reuse past its fp8 cast); then d=0.
                do_proj = cA <= CT - 3
                psws = []
                for p in range(NP):
                    if cA >= 1:
                        mmA(tbs[1][:, p * TS:(p + 1) * TS],
                            zs[p // 2][:, p % 2, cA - 1, :])
                    mmB(tbs[1][:, p * TS:(p + 1) * TS],
                        zs[p // 2][:, p % 2, cB - 1, :])
                    if do_proj:
                        rk = _wslot(p)[2]
                        for c in (cA, cB):
                            psw = psW.tile([64, BO], F32, tag="sW",
                                           name=f"psw{c}_{p}")
                            nc.tensor.matmul(
                                psw[:rk, :],
                                pbt[:, _pbcol(p):_pbcol(p) + rk],
                                zs[p // 2][:, p % 2, c, :],
                                start=True, stop=True,
                            )
                            psws.append((psw, p, c))
                            if len(psws) >= 3:
                                w, q, cq = psws[len(psws) - 3]
                                wj, wo, qk = _wslot(q)
                                cast(wt[wo:wo + qk, wj, cq, :], w[:qk, :])
                for p in range(NP):
                    mmA(tbs[0][:, p * TS:(p + 1) * TS],
                        zs[p // 2][:, p % 2, cA, :])
                    mmB(tbs[0][:, p * TS:(p + 1) * TS],
                        zs[p // 2][:, p % 2, cB, :])
                if do_proj:
                    for w, q, cq in psws[-2:]:
                        wj, wo, qk = _wslot(q)
                        cast(wt[wo:wo + qk, wj, cq, :], w[:qk, :])

                for c, yps in ((cA, ypsA), (cB, ypsB)):
                    ysb = ypool.tile([TS, BO], F32, tag="ysb",
                                     name=f"ysb{c}")
                    if k == CT // 2:
                        # tail: copy halves on both PSUM-capable engines
                        # and DMA each half as soon as its copy lands
                        nc.vector.tensor_copy(ysb[:, :O], yps[:, :O])
                        nc.gpsimd.dma_start(yp_d[c][:, :O], ysb[:, :O])
                        nc.scalar.copy(ysb[:, O:], yps[:, O:])
                        nc.scalar.dma_start(yp_d[c][:, O:], ysb[:, O:])
                    else:
                        cast(ysb[:], yps[:])
                        (nc.scalar if c % 2 else nc.gpsimd).dma_start(
                            yp_d[c], ysb[:])

    _prune_init_barrier(nc)
    _split_sync_waits(nc)
    return nc


# ---------------------------------------------------------------------------
# Host side: input staging, sharding, gather
# ---------------------------------------------------------------------------

def _build_toeplitz(phi_eff):
    """tb[g, d, t, l] = phi_eff[g, d*TS + l - t] (0 where the index is
    negative); phi_eff is [G, L]."""
    G = phi_eff.shape[0]
    pad = np.zeros((G, L + TS), np.float32)
    pad[:, TS:] = phi_eff
    d = np.arange(CT)[:, None, None]
    t = np.arange(TS)[None, :, None]
    l = np.arange(TS)[None, None, :]
    return pad[:, d * TS + l - t + TS]


def _prepare(x, phi, M_phi_plus, M_phi_minus):
    """Host prep: build per-core in_maps (no device execution)."""
    import ml_dtypes

    x = np.asarray(x, np.float32)
    phi = np.asarray(phi, np.float32)
    Mp = np.asarray(M_phi_plus, np.float32)
    Mm = np.asarray(M_phi_minus, np.float32)

    # [i, cp, (b, ic, t)]: x^T, i-major so multi-tile DMA chunks are
    # contiguous per partition
    xT = np.ascontiguousarray(
        x.reshape(B, CT, TS, 2, TS).transpose(4, 1, 0, 3, 2)
    ).reshape(TS, CT, B * 2 * TS)
    sgn = ((-1.0) ** np.arange(L)).astype(np.float32)

    phi_eff = np.empty((2 * K, L), np.float32)
    for g in range(2 * K):
        k, s = g // 2, g % 2
        phi_eff[g] = phi[:, k] * (sgn if s else 1.0)
    tb_all = _build_toeplitz(phi_eff)  # [48, CT, TS, TS]

    # shared far-field t-basis over all pairs' far blocks (d >= 2)
    far_blocks = tb_all[:, 2:]
    gram = np.einsum("gdtl,gdsl->ts", far_blocks.astype(np.float64),
                     far_blocks.astype(np.float64))
    _, evec = np.linalg.eigh(gram)
    P = np.ascontiguousarray(evec[:, ::-1][:, :64]).astype(np.float32)
    G_all = np.einsum("tr,gdtl->gdrl", P, far_blocks)  # [48, 14, 64, TS]
    # per-(pair, r) fp8 range balancing: G rows scaled by alpha, the
    # basis columns (and so W rows) by 1/alpha
    maxG = np.abs(G_all).max(axis=(1, 3))  # [48, 64]
    alpha = np.clip(np.sqrt(8.0 / np.maximum(maxG, 1e-8)), 1.0 / 16, 8.0)
    G_s = G_all * alpha[:, None, :, None]

    nc = _build_nc()
    in_maps = []
    for core in range(N_CORES):
        gs = slice(core * NP, (core + 1) * NP)
        m_core = np.empty((NPP, 2, TS, 2 * O), np.float32)
        for j in range(NPP):
            k = NPP * core + j
            m_core[j, :, :, :O] = Mp[k].reshape(2, TS, O)
            m_core[j, :, :, O:] = Mm[k].reshape(2, TS, O)
        tb_core = np.ascontiguousarray(
            tb_all[gs, :2].transpose(1, 2, 0, 3)
        ).reshape(2, TS, NP * TS)
        g_core = G_s[gs]  # [6, 14, 64, TS]
        a_core = alpha[gs]  # [6, 64]
        pb_core = np.zeros((TS, RW), np.float32)
        gf_core = np.zeros((ND, TS, 2, TS), np.float32)
        for p in range(NP):
            wj, wo, rk = _wslot(p)
            c0 = _pbcol(p)
            pb_core[:, c0:c0 + rk] = P[:, :rk] / a_core[p, :rk]
            gf_core[:, wo:wo + rk, wj, :] = g_core[p, :, :rk, :]
        in_maps.append({
            "xT": xT.astype(ml_dtypes.bfloat16),
            "m": m_core.astype(ml_dtypes.bfloat16),
            "tb": tb_core.astype(ml_dtypes.bfloat16),
            "pb": pb_core.astype(ml_dtypes.bfloat16),
            "gf": gf_core.astype(ml_dtypes.float8_e4m3),
        })
    return nc, in_maps


def _gather(results):
    y = np.zeros((CT, TS, B, O), np.float64)
    for core in range(N_CORES):
        y += results[core]["yp"].reshape(CT, TS, B, O)
    return np.ascontiguousarray(
        y.transpose(2, 0, 1, 3).reshape(B, L, O)
    ).astype(np.float32)


def kernel(x, phi, M_phi_plus, M_phi_minus):
    nc, in_maps = _prepare(x, phi, M_phi_plus, M_phi_minus)
    res = run_bass_kernel_spmd(nc, in_maps, list(range(N_CORES)))
    return _gather(res.results)
